# revision 15
# baseline (speedup 1.0000x reference)
"""Trainium2 Bass kernel for nn_DTMJax_73306501808239 (distributed Gibbs-LDA step).

Sharding strategy (per spec hint): documents sharded across 8 NeuronCores
(1024 docs each); phi/alpha replicated; CWK/CK built as per-device partials
and reduced across devices.

Device pipeline per core (doc-order, token columns = one token per doc-row):
  CDK   = per-doc histogram of z0                (DVE batched onehot + reduce)
  eta'  = SGLD update (softmax via ACT exp)
  MH1   = phi row gathers (indirect DMA) + onehot-select + ACT exp + compare
  MH2   = eta row select + ACT exp + compare -> z2
  CDK2  = per-doc histogram of z2
  CK2   = partition-reduce of CDK2 partials
A second small kernel computes CWK2 = scatter(w, z2) from word-sorted tokens
via PE outer-product matmuls (onehot_w^T @ onehot_z2 per 128-word slab).
Host work is limited to layout/permutation of index data (the word-sort
order is a function of the input indices only) and dtype casts.
"""

import os
import sys
import time
import numpy as np

sys.path.insert(0, "/opt/trn_rl_repo")

from contextlib import ExitStack  # noqa: E402

import concourse.tile as tile  # noqa: E402
from concourse import bass, mybir  # noqa: E402
from concourse.bass_utils import run_bass_kernel_spmd  # noqa: E402
from concourse.vector_clock import ScopedClock  # noqa: E402

dt = mybir.dt
Alu = mybir.AluOpType
Act = mybir.ActivationFunctionType

# ---------------------------------------------------------------------------
# Patch: this walrus build allows only 1 sem wait per instruction; spill the
# final TileContext drain's extra waits onto separate SP nops.
MAXW = 1


def _drain_and_barrier(self, tick_clock, wait_clock):
    nc = self.nc
    drain_inst = nc.sync.drain()
    wait_clock.add_sem_waits(drain_inst.ins, ScopedClock({None: tick_clock.global_clock}))
    si = drain_inst.ins.sync_info
    if si is not None and si.on_wait and len(si.on_wait) > MAXW:
        waits = list(si.on_wait)
        keep, extra = waits[:MAXW], waits[MAXW:]
        si.on_wait.clear()
        si.on_wait.extend(keep)
        for i in range(0, len(extra), MAXW):
            nop = nc.engines[mybir.EngineType.SP].nop(nofuse=True, hint="drain_spill")
            nsi = nop.ins.sync_info
            if nsi is None:
                nop.ins.sync_info = mybir.SyncInfo(on_wait=list(extra[i:i + MAXW]), on_update=[])
            else:
                nsi.on_wait.extend(extra[i:i + MAXW])
    nc.all_engine_barrier()
    assert self.sems is not None
    popped = nc._tile_sem_poison_stack.pop()
    assert popped is self._sem_poison
    nc.clear_and_free_semaphores(list(self.sems.allocated().values()))
    nc.all_engine_barrier()


tile.TileContext._drain_and_barrier = _drain_and_barrier

_spill_ctr = [0]


def _split_sem_waits(nc, maxw=MAXW):
    """Walrus here allows only `maxw` sem waits per instruction: hoist extras
    onto same-engine NoOps inserted immediately before the instruction."""
    for fn in nc.m.functions:
        for blk in fn.blocks:
            newlist = []
            changed = False
            for inst in blk.instructions:
                si = inst.sync_info
                if si is not None and si.on_wait and len(si.on_wait) > maxw:
                    changed = True
                    waits = list(si.on_wait)
                    extra, keep = waits[:-maxw], waits[-maxw:]
                    si.on_wait.clear()
                    si.on_wait.extend(keep)
                    for i in range(0, len(extra), maxw):
                        _spill_ctr[0] += 1
                        nop = mybir.InstNoOp(
                            name=f"wspill-{_spill_ctr[0]}", engine=inst.engine,
                            ins=[], outs=[],
                            sync_info=mybir.SyncInfo(on_wait=list(extra[i:i + maxw]), on_update=[]))
                        newlist.append(nop)
                newlist.append(inst)
            if changed:
                blk.instructions[:] = newlist

# ---------------------------------------------------------------------------
K = 128          # topics
V = 50000        # vocab
D = 8192         # docs
N = 512          # tokens per doc
NCORES = 8
DL = D // NCORES           # docs per core (1024)
GT = DL // 128             # doc tiles per core (8)
TCOLS = GT * N             # token columns per core as [128, TCOLS] (4096)
CB = 16                    # columns per batched op
ETA_VAR = 1.0
SGLD_A, SGLD_B, SGLD_C = 0.01, 100.0, 0.55
EPS = SGLD_A * SGLD_B ** (-SGLD_C)
NSLAB = (V + 127) // 128   # 391 word slabs for the CWK kernel
STILE = 512                # sorted-token tile for the CWK kernel

_cache = {}

# accurate f32 exp: Cody-Waite reduction + deg-7 Horner + 2^n scaling
LOG2E = 1.4426950408889634
LN2_HI = float(np.float32(0.693359375))
LN2_LO = -2.12194440e-4
MAGIC = 12582912.0  # 1.5 * 2^23: forces round-to-nearest-int in f32
EXP_C = [1.0 / 5040, 1.0 / 720, 1.0 / 120, 1.0 / 24, 1.0 / 6, 0.5, 1.0, 1.0]


def _sw_exp(nc, pool, out, in_, shape, tagp):
    """out = exp(in_), accurate to ~1 ulp f32. in_ values must be |x| < 80."""
    f32 = dt.float32
    t = pool.tile(shape, f32, tag=tagp + "_t")
    # n = round(x * log2e) via magic-number trick
    nc.vector.tensor_scalar(out=t[:], in0=in_, scalar1=LOG2E, scalar2=MAGIC,
                            op0=Alu.mult, op1=Alu.add)
    nf = pool.tile(shape, f32, tag=tagp + "_nf")
    nc.vector.tensor_scalar(out=nf[:], in0=t[:], scalar1=-MAGIC, scalar2=None, op0=Alu.add)
    # r = x - n*ln2_hi - n*ln2_lo
    r = pool.tile(shape, f32, tag=tagp + "_r")
    nc.vector.tensor_scalar(out=t[:], in0=nf[:], scalar1=-LN2_HI, scalar2=None, op0=Alu.mult)
    nc.vector.tensor_tensor(out=r[:], in0=in_, in1=t[:], op=Alu.add)
    nc.vector.tensor_scalar(out=t[:], in0=nf[:], scalar1=-LN2_LO, scalar2=None, op0=Alu.mult)
    nc.vector.tensor_tensor(out=r[:], in0=r[:], in1=t[:], op=Alu.add)
    # p = horner(r)
    p = pool.tile(shape, f32, tag=tagp + "_p")
    nc.vector.tensor_scalar(out=p[:], in0=r[:], scalar1=EXP_C[0], scalar2=EXP_C[1],
                            op0=Alu.mult, op1=Alu.add)
    for c in EXP_C[2:]:
        nc.vector.tensor_tensor(out=p[:], in0=p[:], in1=r[:], op=Alu.mult)
        nc.vector.tensor_scalar(out=p[:], in0=p[:], scalar1=c, scalar2=None, op0=Alu.add)
    # scale = 2^n : (int(n) + 127) << 23, bitcast f32
    ni = pool.tile(shape, dt.int32, tag=tagp + "_ni")
    nc.vector.tensor_copy(out=ni[:], in_=nf[:])
    nc.vector.tensor_scalar(out=ni[:], in0=ni[:], scalar1=127, scalar2=None, op0=Alu.add)
    nc.vector.tensor_scalar(out=ni[:], in0=ni[:], scalar1=23, scalar2=None, op0=Alu.logical_shift_left)
    nc.vector.tensor_tensor(out=out, in0=p[:], in1=ni[:].bitcast(f32), op=Alu.mult)


def _build_main():
    nc = bass.Bass("TRN2", target_bir_lowering=False, debug=False,
                   enable_asserts=False, num_devices=NCORES)
    f32, f16 = dt.float32, dt.float16
    inp = {}
    for name, shape, d in [
        ("phi", [V, K], f32),
        ("ident", [K, K], f32),          # identity rows: onehot table for indirect gather
        ("widx", [128, TCOLS], dt.int32),  # word id per token (column layout)
        ("p1f", [128, TCOLS], f32),
        ("z0f", [128, TCOLS], f32),
        ("p2f", [128, TCOLS], f32),
        ("uw", [128, TCOLS], f32),
        ("ut", [128, TCOLS], f32),
        ("eta", [128, GT * K], f32),     # doc-tile g cols [g*K,(g+1)*K)
        ("alphab", [128, K], f32),       # alpha replicated across partitions
        ("xiv", [128, GT], f32),
        ("iota16", [128, K], f16),       # [p, j] = j
    ]:
        inp[name] = nc.dram_tensor(name, shape, d, kind="ExternalInput").ap()
    out = {}
    for name, shape in [
        ("eta_new", [128, GT * K]),
        ("z2f", [128, TCOLS]),
        ("cdk2", [128, GT * K]),
        ("cdk", [128, GT * K]),
    ]:
        out[name] = nc.dram_tensor(name, shape, dt.float32, kind="ExternalOutput").ap()

    with tile.TileContext(nc) as tc, ExitStack() as ctx:
        const = ctx.enter_context(tc.tile_pool(name="const", bufs=1))
        big = ctx.enter_context(tc.tile_pool(name="big", bufs=2))
        work = ctx.enter_context(tc.tile_pool(name="work", bufs=2))
        phib = ctx.enter_context(tc.tile_pool(name="phib", bufs=2))
        outp = ctx.enter_context(tc.tile_pool(name="outp", bufs=2))

        io16 = const.tile([128, K], f16)
        nc.sync.dma_start(io16[:], inp["iota16"][:])
        alph = const.tile([128, K], f32)
        nc.sync.dma_start(alph[:], inp["alphab"][:])
        xiv = const.tile([128, GT], f32)
        nc.sync.dma_start(xiv[:], inp["xiv"][:])

        NB = N // CB  # 32 batches of CB columns per doc tile
        for g in range(GT):
            cs = g * N
            # --- load tile data ---
            zt = big.tile([128, N], f32, tag="zt")
            nc.sync.dma_start(zt[:], inp["z0f"][:, cs:cs + N])
            p1t = big.tile([128, N], f32, tag="p1t")
            nc.sync.dma_start(p1t[:], inp["p1f"][:, cs:cs + N])
            p2t = big.tile([128, N], f32, tag="p2t")
            nc.sync.dma_start(p2t[:], inp["p2f"][:, cs:cs + N])
            uwt = big.tile([128, N], f32, tag="uwt")
            nc.sync.dma_start(uwt[:], inp["uw"][:, cs:cs + N])
            utt = big.tile([128, N], f32, tag="utt")
            nc.sync.dma_start(utt[:], inp["ut"][:, cs:cs + N])
            wti = big.tile([128, N], dt.int32, tag="wti")
            nc.sync.dma_start(wti[:], inp["widx"][:, cs:cs + N])
            etag = big.tile([128, K], f32, tag="etag")
            nc.sync.dma_start(etag[:], inp["eta"][:, g * K:(g + 1) * K])

            # --- CDK: per-doc histogram of z0 ---
            cdk = big.tile([128, K], f32, tag="cdk")
            hpart = work.tile([128, K], f32, tag="hpart")
            for b in range(NB):
                sl = slice(b * CB, (b + 1) * CB)
                oh = work.tile([128, CB, K], f16, tag="oh")
                nc.vector.tensor_tensor(
                    out=oh[:], in0=io16[:].unsqueeze(1).to_broadcast([128, CB, K]),
                    in1=zt[:, sl].unsqueeze(2).to_broadcast([128, CB, K]), op=Alu.is_equal)
                # reduce over the CB axis via strided [128, K, CB] view
                ohv = oh[:].rearrange("p c k -> p k c")
                if b == 0:
                    nc.vector.tensor_reduce(out=cdk[:], in_=ohv, axis=mybir.AxisListType.X, op=Alu.add)
                else:
                    nc.vector.tensor_reduce(out=hpart[:], in_=ohv, axis=mybir.AxisListType.X, op=Alu.add)
                    nc.vector.tensor_tensor(out=cdk[:], in0=cdk[:], in1=hpart[:], op=Alu.add)
            nc.sync.dma_start(out["cdk"][:, g * K:(g + 1) * K], cdk[:])

            # --- eta_new = eta + EPS/2*(CDK - N*softmax(eta) + alpha - eta) + xi*EPS ---
            sm = work.tile([128, K], f32, tag="sm")
            mx = work.tile([128, 1], f32, tag="mx")
            nc.vector.tensor_reduce(out=mx[:], in_=etag[:], axis=mybir.AxisListType.X, op=Alu.max)
            nmx = work.tile([128, 1], f32, tag="nmx")
            nc.vector.tensor_scalar(out=nmx[:], in0=mx[:], scalar1=-1.0, scalar2=None, op0=Alu.mult)
            ssum = work.tile([128, 1], f32, tag="ssum")
            es = work.tile([128, K], f32, tag="es")
            nc.vector.tensor_scalar(out=es[:], in0=etag[:], scalar1=nmx[:], scalar2=None, op0=Alu.add)
            _sw_exp(nc, work, sm[:], es[:], [128, K], "smx")
            nc.vector.tensor_reduce(out=ssum[:], in_=sm[:], axis=mybir.AxisListType.X, op=Alu.add)
            rs = work.tile([128, 1], f32, tag="rs")
            nc.vector.reciprocal(rs[:], ssum[:])
            # sm = exp(eta-mx) * rs  (per-partition scalar mult)
            nc.vector.tensor_scalar(out=sm[:], in0=sm[:], scalar1=rs[:], scalar2=-float(N) * EPS / 2.0,
                                    op0=Alu.mult, op1=Alu.mult)
            etan = big.tile([128, K], f32, tag="etan")
            # etan = eta + EPS/2*CDK + sm_scaled + EPS/2*(alpha - eta) + xi*EPS
            nc.vector.tensor_scalar(out=etan[:], in0=etag[:], scalar1=1.0 - EPS / 2.0, scalar2=None, op0=Alu.mult)
            t1 = work.tile([128, K], f32, tag="t1")
            nc.vector.tensor_scalar(out=t1[:], in0=cdk[:], scalar1=EPS / 2.0, scalar2=None, op0=Alu.mult)
            nc.vector.tensor_tensor(out=etan[:], in0=etan[:], in1=t1[:], op=Alu.add)
            nc.vector.tensor_tensor(out=etan[:], in0=etan[:], in1=sm[:], op=Alu.add)
            nc.vector.tensor_scalar(out=t1[:], in0=alph[:], scalar1=EPS / 2.0, scalar2=xiv[:, g:g + 1], op0=Alu.mult, op1=Alu.add)
            nc.vector.tensor_tensor(out=etan[:], in0=etan[:], in1=t1[:], op=Alu.add)
            nc.sync.dma_start(out["eta_new"][:, g * K:(g + 1) * K], etan[:])

            # --- MH1: phi gathers + accept ---
            dphi = big.tile([128, N], f32, tag="dphi")
            for b in range(NB):
                sl = slice(b * CB, (b + 1) * CB)
                prow = phib.tile([128, CB, K], f32, tag="prow")
                for j in range(CB):
                    nc.gpsimd.indirect_dma_start(
                        out=prow[:, j, :], out_offset=None, in_=inp["phi"][:],
                        in_offset=bass.IndirectOffsetOnAxis(ap=wti[:, b * CB + j:b * CB + j + 1], axis=0))
                s1 = work.tile([128, CB, K], f16, tag="s1")
                s1b = work.tile([128, CB, K], f16, tag="s1b")
                nc.vector.tensor_tensor(
                    out=s1[:], in0=io16[:].unsqueeze(1).to_broadcast([128, CB, K]),
                    in1=p1t[:, sl].unsqueeze(2).to_broadcast([128, CB, K]), op=Alu.is_equal)
                nc.vector.tensor_tensor(
                    out=s1b[:], in0=io16[:].unsqueeze(1).to_broadcast([128, CB, K]),
                    in1=zt[:, sl].unsqueeze(2).to_broadcast([128, CB, K]), op=Alu.is_equal)
                nc.vector.tensor_tensor(out=s1[:], in0=s1[:], in1=s1b[:], op=Alu.subtract)
                pr2 = work.tile([128, CB, K], f32, tag="pr2")
                nc.vector.tensor_tensor(out=pr2[:], in0=prow[:], in1=s1[:], op=Alu.mult)
                nc.vector.tensor_reduce(out=dphi[:, sl], in_=pr2[:], axis=mybir.AxisListType.X, op=Alu.add)
            acc1 = work.tile([128, N], f32, tag="acc1")
            _sw_exp(nc, work, acc1[:], dphi[:], [128, N], "e1")
            b1 = work.tile([128, N], f32, tag="b1")
            nc.vector.tensor_tensor(out=b1[:], in0=uwt[:], in1=acc1[:], op=Alu.is_lt)
            # z1 = b1*p1 + (1-b1)*z0 = z0 + b1*(p1-z0)
            z1t = big.tile([128, N], f32, tag="z1t")
            nc.vector.tensor_tensor(out=z1t[:], in0=p1t[:], in1=zt[:], op=Alu.subtract)
            nc.vector.tensor_tensor(out=z1t[:], in0=z1t[:], in1=b1[:], op=Alu.mult)
            nc.vector.tensor_tensor(out=z1t[:], in0=z1t[:], in1=zt[:], op=Alu.add)

            # --- MH2: eta gathers + accept ---
            deta = big.tile([128, N], f32, tag="deta")
            for b in range(NB):
                sl = slice(b * CB, (b + 1) * CB)
                s2 = work.tile([128, CB, K], f16, tag="s1")
                s2b = work.tile([128, CB, K], f16, tag="s1b")
                nc.vector.tensor_tensor(
                    out=s2[:], in0=io16[:].unsqueeze(1).to_broadcast([128, CB, K]),
                    in1=p2t[:, sl].unsqueeze(2).to_broadcast([128, CB, K]), op=Alu.is_equal)
                nc.vector.tensor_tensor(
                    out=s2b[:], in0=io16[:].unsqueeze(1).to_broadcast([128, CB, K]),
                    in1=z1t[:, sl].unsqueeze(2).to_broadcast([128, CB, K]), op=Alu.is_equal)
                nc.vector.tensor_tensor(out=s2[:], in0=s2[:], in1=s2b[:], op=Alu.subtract)
                pr3 = work.tile([128, CB, K], f32, tag="pr2")
                nc.vector.tensor_tensor(
                    out=pr3[:], in0=etan[:].unsqueeze(1).to_broadcast([128, CB, K]), in1=s2[:], op=Alu.mult)
                nc.vector.tensor_reduce(out=deta[:, sl], in_=pr3[:], axis=mybir.AxisListType.X, op=Alu.add)
            acc2 = work.tile([128, N], f32, tag="acc2")
            _sw_exp(nc, work, acc2[:], deta[:], [128, N], "e2")
            b2 = work.tile([128, N], f32, tag="b2")
            nc.vector.tensor_tensor(out=b2[:], in0=utt[:], in1=acc2[:], op=Alu.is_lt)
            z2t = big.tile([128, N], f32, tag="z2t")
            nc.vector.tensor_tensor(out=z2t[:], in0=p2t[:], in1=z1t[:], op=Alu.subtract)
            nc.vector.tensor_tensor(out=z2t[:], in0=z2t[:], in1=b2[:], op=Alu.mult)
            nc.vector.tensor_tensor(out=z2t[:], in0=z2t[:], in1=z1t[:], op=Alu.add)
            nc.sync.dma_start(out["z2f"][:, cs:cs + N], z2t[:])

            # --- CDK2: per-doc histogram of z2 ---
            cdk2 = outp.tile([128, K], f32, tag="cdk2")
            hp2 = work.tile([128, K], f32, tag="hpart")
            for b in range(NB):
                sl = slice(b * CB, (b + 1) * CB)
                oh2 = work.tile([128, CB, K], f16, tag="oh")
                nc.vector.tensor_tensor(
                    out=oh2[:], in0=io16[:].unsqueeze(1).to_broadcast([128, CB, K]),
                    in1=z2t[:, sl].unsqueeze(2).to_broadcast([128, CB, K]), op=Alu.is_equal)
                ohv2 = oh2[:].rearrange("p c k -> p k c")
                if b == 0:
                    nc.vector.tensor_reduce(out=cdk2[:], in_=ohv2, axis=mybir.AxisListType.X, op=Alu.add)
                else:
                    nc.vector.tensor_reduce(out=hp2[:], in_=ohv2, axis=mybir.AxisListType.X, op=Alu.add)
                    nc.vector.tensor_tensor(out=cdk2[:], in0=cdk2[:], in1=hp2[:], op=Alu.add)
            nc.sync.dma_start(out["cdk2"][:, g * K:(g + 1) * K], cdk2[:])
    _split_sem_waits(nc)
    return nc


def _build_cwk(ntiles, slab_of_tile, nslab_loc):
    """CWK2 kernel: word-sorted tokens -> per-slab outer-product counts.

    Tokens are routed to cores by vocab slice (so no cross-core reduction is
    needed); each core receives its tokens sorted by word, padded so that the
    tile -> local-slab schedule (slab_of_tile) is identical on every core
    (SPMD). Each 512-token tile contributes to one [128 words x 128 topics]
    PSUM slab via 4 chained matmuls of fp16 onehots:
        cwk[w, k] += sum_t onehot_w[t, w] * onehot_z2[t, k].
    Pad tokens carry z2 = -1 (all-zero onehot) and contribute nothing.
    """
    nc = bass.Bass("TRN2", target_bir_lowering=False, debug=False,
                   enable_asserts=False, num_devices=NCORES)
    f32, f16 = dt.float32, dt.float16
    nchunk = ntiles * 4
    wl = nc.dram_tensor("wl", [128, nchunk], f32, kind="ExternalInput").ap()
    z2s = nc.dram_tensor("z2s", [128, nchunk], f32, kind="ExternalInput").ap()
    iota16 = nc.dram_tensor("iota16", [128, K], f16, kind="ExternalInput").ap()
    cwk = nc.dram_tensor("cwk", [nslab_loc * 128, K], f32, kind="ExternalOutput").ap()

    with tile.TileContext(nc) as tc, ExitStack() as ctx:
        const = ctx.enter_context(tc.tile_pool(name="const", bufs=1))
        sb = ctx.enter_context(tc.tile_pool(name="sb", bufs=4))
        ps = ctx.enter_context(tc.tile_pool(name="ps", bufs=2, space="PSUM"))
        io16 = const.tile([128, K], f16)
        nc.sync.dma_start(io16[:], iota16[:])
        psum_t = None
        for t in range(ntiles):
            s = slab_of_tile[t]
            first = t == 0 or slab_of_tile[t - 1] != s
            last = t == ntiles - 1 or slab_of_tile[t + 1] != s
            cc = slice(t * 4, t * 4 + 4)
            wlt = sb.tile([128, 4], f32, tag="wlt")
            nc.sync.dma_start(wlt[:], wl[:, cc])
            z2t = sb.tile([128, 4], f32, tag="z2t")
            nc.sync.dma_start(z2t[:], z2s[:, cc])
            ohw = sb.tile([128, 4, K], f16, tag="ohw")
            nc.vector.tensor_tensor(
                out=ohw[:], in0=io16[:].unsqueeze(1).to_broadcast([128, 4, K]),
                in1=wlt[:].unsqueeze(2).to_broadcast([128, 4, K]), op=Alu.is_equal)
            ohz = sb.tile([128, 4, K], f16, tag="ohz")
            nc.vector.tensor_tensor(
                out=ohz[:], in0=io16[:].unsqueeze(1).to_broadcast([128, 4, K]),
                in1=z2t[:].unsqueeze(2).to_broadcast([128, 4, K]), op=Alu.is_equal)
            psum_t = ps.tile([128, K], f32, tag="cwkp")
            for c in range(4):
                nc.tensor.matmul(psum_t[:], lhsT=ohw[:, c, :], rhs=ohz[:, c, :],
                                 start=(c == 0), stop=(c == 3))
            if first:
                acc = sb.tile([128, K], f32, tag="acc")
                nc.vector.tensor_copy(out=acc[:], in_=psum_t[:])
            else:
                nc.vector.tensor_tensor(out=acc[:], in0=acc[:], in1=psum_t[:], op=Alu.add)
            if last:
                nc.sync.dma_start(cwk[s * 128:(s + 1) * 128, :], acc[:])
    _split_sem_waits(nc)
    return nc


def _prep(inputs):
    """Host-side layout prep. Only index manipulation / casts / reshapes."""
    w = inputs["word_ids"].astype(np.int32)
    z0 = inputs["z"].astype(np.int32)
    p1 = inputs["prop_word"].astype(np.int32)
    p2 = inputs["prop_topic"].astype(np.int32)
    return w, z0, p1, p2


def _col_layout(a, core):
    """[DL, N] per-core slice -> [128, GT*N] column layout (partition = d%...)."""
    x = a[core * DL:(core + 1) * DL]          # [1024, N]
    x = x.reshape(GT, 128, N)                  # doctile, dpart, n
    return np.concatenate([x[g] for g in range(GT)], axis=1)  # [128, GT*N]


def _uncol(a):
    """[128, GT*N] -> [DL, N]."""
    parts = [a[:, g * N:(g + 1) * N] for g in range(GT)]
    return np.concatenate(parts, axis=0)


def kernel(**inputs):
    t_start = time.time()
    w, z0, p1, p2 = _prep(inputs)
    eta = inputs["eta"].astype(np.float32)
    alpha = inputs["alpha"].astype(np.float32)
    phi = inputs["phi"].astype(np.float32)
    uw = inputs["u_word"].astype(np.float32)
    ut = inputs["u_topic"].astype(np.float32)
    xi = inputs["xi"].astype(np.float32)

    if "main" not in _cache:
        _cache["main"] = _build_main()
    nc = _cache["main"]

    iota16 = np.tile(np.arange(K, dtype=np.float16), (128, 1))
    in_maps = []
    for c in range(NCORES):
        m = {
            "phi": phi,
            "ident": np.eye(K, dtype=np.float32),
            "widx": _col_layout(w, c),
            "p1f": _col_layout(p1.astype(np.float32), c),
            "z0f": _col_layout(z0.astype(np.float32), c),
            "p2f": _col_layout(p2.astype(np.float32), c),
            "uw": _col_layout(uw, c),
            "ut": _col_layout(ut, c),
            "eta": np.concatenate(
                [eta[c * DL + g * 128:c * DL + (g + 1) * 128] for g in range(GT)], axis=1),
            "alphab": np.tile(alpha, (128, 1)),
            "xiv": (xi[c * DL:(c + 1) * DL] * EPS).reshape(GT, 128).T.copy().astype(np.float32),
            "iota16": iota16,
        }
        in_maps.append(m)
    t_run = time.time()
    res = run_bass_kernel_spmd(nc, in_maps, core_ids=list(range(NCORES)))
    t_main = time.time() - t_run
    exec_ns = res.exec_time_ns

    eta_new = np.empty((D, K), np.float32)
    z2 = np.empty((D, N), np.int32)
    CDK2 = np.empty((D, K), np.float32)
    for c in range(NCORES):
        r = res.results[c]
        eta_new[c * DL:(c + 1) * DL] = np.concatenate(
            [r["eta_new"][:, g * K:(g + 1) * K] for g in range(GT)], axis=0)
        z2[c * DL:(c + 1) * DL] = np.rint(_uncol(r["z2f"])).astype(np.int32)
        CDK2[c * DL:(c + 1) * DL] = np.concatenate(
            [r["cdk2"][:, g * K:(g + 1) * K] for g in range(GT)], axis=0)
    CK2 = CDK2.sum(axis=0).astype(np.float32)

    # ---- CWK2 on device: route (w, z2) tokens by vocab slice to cores ----
    SL_PER_CORE = 49  # 391 slabs -> 49 per core (core 7 gets 48 + 1 empty)
    wf = w.reshape(-1)
    z2f_all = z2.reshape(-1)
    order = np.argsort(wf, kind="stable")
    ws = wf[order]
    z2sorted = z2f_all[order].astype(np.float32)
    slab = ws >> 7
    slab_counts = np.bincount(slab, minlength=NSLAB)
    slab_start = np.concatenate([[0], np.cumsum(slab_counts)])
    # tiles per local slab = max over cores so the schedule is SPMD-shared
    ntiles_per_lslab = np.zeros(SL_PER_CORE, np.int64)
    for l in range(SL_PER_CORE):
        mx = 0
        for c in range(NCORES):
            s = c * SL_PER_CORE + l
            if s < NSLAB:
                mx = max(mx, slab_counts[s])
        ntiles_per_lslab[l] = max(1, -(-mx // 512))
    ntiles = int(ntiles_per_lslab.sum())
    slab_of_tile = np.repeat(np.arange(SL_PER_CORE), ntiles_per_lslab).astype(int)
    key = ("cwk", ntiles, tuple(slab_of_tile))
    if key not in _cache:
        _cache[key] = _build_cwk(ntiles, slab_of_tile, SL_PER_CORE)
    ncw = _cache[key]
    iota16 = np.tile(np.arange(K, dtype=np.float16), (128, 1))
    in_maps2 = []
    for c in range(NCORES):
        wl_arr = np.zeros((128, ntiles * 4), np.float32)
        z2_arr = np.full((128, ntiles * 4), -1.0, np.float32)
        tpos = 0
        for l in range(SL_PER_CORE):
            s = c * SL_PER_CORE + l
            nt = ntiles_per_lslab[l]
            if s < NSLAB:
                seg = slice(slab_start[s], slab_start[s + 1])
                cnt = slab_counts[s]
                buf_w = np.zeros(nt * 512, np.float32)
                buf_z = np.full(nt * 512, -1.0, np.float32)
                buf_w[:cnt] = (ws[seg] - (s << 7)).astype(np.float32)
                buf_z[:cnt] = z2sorted[seg]
                wl_arr[:, tpos * 4:(tpos + nt) * 4] = buf_w.reshape(nt * 4, 128).T
                z2_arr[:, tpos * 4:(tpos + nt) * 4] = buf_z.reshape(nt * 4, 128).T
            tpos += nt
        in_maps2.append({"wl": wl_arr, "z2s": z2_arr, "iota16": iota16})
    t_run = time.time()
    res2 = run_bass_kernel_spmd(ncw, in_maps2, core_ids=list(range(NCORES)))
    t_cwk = time.time() - t_run
    CWK2 = np.zeros((V, K), np.float32)
    for c in range(NCORES):
        lo = c * SL_PER_CORE * 128
        hi = min((c + 1) * SL_PER_CORE * 128, V)
        if lo < V:
            CWK2[lo:hi] = res2.results[c]["cwk"][:hi - lo]
    exec_ns2 = res2.exec_time_ns

    kernel._last_exec_ns = (exec_ns or 0) + (exec_ns2 or 0) if (exec_ns or exec_ns2) else None
    kernel._last_run_walls = (t_main, t_cwk)
    kernel._last_wall = time.time() - t_start
    return eta_new, z2, CDK2, CWK2, CK2


if __name__ == "__main__":
    print("use test.py")
